# revision 40
# baseline (speedup 1.0000x reference)
"""Trainium2 Bass kernel for nn_Attention_XL (B=2,T=2048,C=1024,S=2048,H=16).

Sharding: (batch, head) pairs across 8 cores — core c handles batch c//4 and
heads [(c%4)*4, (c%4)*4+4). QKV projection is column-sharded by head (no
redundant FLOPs); W_proj is column-sharded; per-core partial outputs are
summed on the host (the tensor-parallel unshard step).

All DMA-facing tensors are bf16 (host converts; pos_emb is pre-added into
the k_xl cache on the host): measured DMA bandwidth into SBUF collapses
~7x while the engines are busy, so halving bytes moved is worth far more
than it costs in precision (rel err 6e-3 vs the 2e-2 gate).

Per-core device program (feature-on-partition, i.e. transposed; matmul
operands bf16, PSUM fp32):
  kcat^T = [kx^T (pos pre-added), kn^T]; vn computed directly in [t, dim]
  layout into v_aug, whose ones columns later yield the softmax
  denominator rows.
  Attention per head pair, flash-style over key chunks, software
  pipelined: sc[l] is emitted before attnV[l-1] so the in-order PE stream
  never waits on exp[l] with score work available; ACT (exp, [128,1024]
  per chunk, scale folded in) is the pace-setting engine. et ring bufs=4
  lets ACT run ahead of attnV.
  Chunk-boundary normalization: the PSUM accumulators are copied to SBUF
  immediately (a K=1 broadcast matmul here stalls the PE ~5us, and gpsimd
  partition_broadcast returns zeros on this toolchain, so the sumexp row
  is partition-broadcast via a DRAM-roundtrip DMA on the gpsimd queue);
  reciprocal+scale then run off the critical path.
  Pair-1's QKV m-tiles and the output projection are drip-fed into the
  ACT-bound l-loops (schedule in drip_sched / proj feed); pair-1's XL
  loads and all mid-loop DMAs ride the gpsimd SWDGE queue to stay off the
  SP queue that serves the prologue streams.
"""
import sys

sys.path.insert(0, "/opt/trn_rl_repo")

import numpy as np
import concourse.bass as bass
import concourse.bacc as bacc
import concourse.mybir as mybir
import concourse.tile as tile
from concourse.bass_utils import run_bass_kernel_spmd

F32 = mybir.dt.float32
F32R = mybir.dt.float32r
BF16 = mybir.dt.bfloat16
AF = mybir.ActivationFunctionType

HD = 64          # head dim
HPC = 4          # heads per core
N_CORES = 8


def r(ap):
    return ap.bitcast(F32R)


def build_program(T, S, C, timing_mode=False, stage="full", repeat=1,
                  pipelined=True):
    """Build + compile the per-core Bass program. Same program on all cores.

    timing_mode: big I/O tensors become Internal DRAM (no host transfer) so
    pipelined wall-clock isolates device exec; compute is unchanged.
    stage: 'full' | 'attn1' (pair-0 attention only) | 'qkv' (no attention) |
    'dma' (loads only) — timing diagnostics, values meaningless off 'full'.
    repeat: emit the whole body k times (timing: marginal cost per repeat
    isolates device exec from per-call submission overhead).
    pipelined: emit attnV[l-1] after sc[l] (software pipelining) vs legacy
    same-iteration order."""
    L = S + T
    nL = L // 128           # key chunks
    nS = S // 128
    nT = T // 128
    nC = C // 128           # contraction chunks for qkv proj
    nTc = T // 512          # 512-wide t chunks
    R = 3 * HPC * HD        # rows of W' (768)
    scale = 1.0 / np.sqrt(HD)

    nc = bacc.Bacc("TRN2", target_bir_lowering=False, debug=False)

    ik = "Internal" if timing_mode else "ExternalInput"
    ok = "Internal" if timing_mode else "ExternalOutput"
    qT = nc.dram_tensor("qT", [C, T], BF16, kind=ik).ap()
    wqkvT = nc.dram_tensor("wqkvT", [C, R], BF16, kind=ik).ap()
    wp4 = nc.dram_tensor("wp4", [128, 2 * C], BF16, kind=ik).ap()
    kxlT = nc.dram_tensor("kxlT", [HPC * HD, S], BF16, kind=ik).ap()
    vxl = nc.dram_tensor("vxl", [S, HPC * HD], BF16, kind=ik).ap()
    n_ones = max(64, 2 * (S + T) // 128)
    ones_in = nc.dram_tensor("ones", [128, n_ones], BF16, kind=ik).ap()
    outT = nc.dram_tensor("outT", [C, T], BF16, kind=ok).ap()
    nscratch = nc.dram_tensor("nscratch", [2, 512], F32, kind="Internal").ap()
    if timing_mode:
        dummy = nc.dram_tensor("tm_in", [128, 128], F32,
                               kind="ExternalInput").ap()
        tiny = nc.dram_tensor("tm_out", [128, 128], F32,
                              kind="ExternalOutput").ap()

    with tile.TileContext(nc) as tc:
        import contextlib
        for _rep in range(repeat):
          with contextlib.ExitStack() as ctx:
            persist = ctx.enter_context(tc.tile_pool(name="persist", bufs=1))
            vaugp = ctx.enter_context(tc.tile_pool(name="vaugp", bufs=1))
            att2 = ctx.enter_context(tc.tile_pool(name="att_sb", bufs=1))

            qnT = [persist.tile([128, T], BF16, tag=f"qnT{p}", name=f"qnT{p}")
                   for p in range(2)]
            kcatT = [persist.tile([128, L], BF16, tag=f"kcatT{p}",
                                  name=f"kcatT{p}") for p in range(2)]
            yT = [persist.tile([128, T], BF16, tag=f"yTp{p}",
                  name=f"yTp{p}") for p in range(2)]
            v_aug = [vaugp.tile([128, nL * 66], BF16, tag=f"vaug{h}",
                                name=f"vaug{h}") for h in range(HPC)]

            # psum pools for the attention loops are entered mid-build
            # (after the big qkv pool closes); declared here for closures
            sc_ps = mm2_ps = pj_ps = None
            proj_group = None

            def normalize(hA, hB, oA, oB, ts):
                # phase 1 (chunk boundary): copy the PSUM accumulators to
                # SBUF so the next chunk's attnV start=True isn't gated on
                # the whole normalization chain. A K=1 broadcast matmul here
                # measurably stalls the PE (~5us each) - avoided entirely.
                ocpA = att2.tile([66, 512], F32, tag="ocp", bufs=4,
                                 name="ocpA")
                nc.vector.tensor_copy(r(ocpA[:]), oA[:])
                ocpB = att2.tile([66, 512], F32, tag="ocp", bufs=4,
                                 name="ocpB")
                nc.vector.tensor_copy(r(ocpB[:]), oB[:])
                return (hA, hB, ocpA, ocpB, ts)

            def normalize2(hA, hB, oA, oB, ts):
                # phase 2 (deferred): y^T = out^T * (1/sumexp). The sumexp
                # row is partition-broadcast via a DRAM-roundtrip DMA (the
                # K=1 matmul alternative stalls the PE ~5us; gpsimd
                # partition_broadcast returns zeros on this toolchain).
                # Heads are paired into one [128,T] y tile so the output
                # projection contracts K=128: head A lands in rows 0:64
                # directly; head B goes through a temp and a repartitioning
                # SBUF->SBUF DMA into rows 64:128 (DVE can't cross
                # partitions).
                pr = hA // 2
                for j, (h, o) in enumerate(((hA, oA), (hB, oB))):
                    nc.gpsimd.dma_start(nscratch[j:j + 1, :], o[64:65, :])
                    bcast = att2.tile([64, 512], F32, tag="bcast", bufs=2,
                                      name="bcast")
                    nc.gpsimd.dma_start(
                        bcast[:], nscratch[j:j + 1, :].broadcast_to([64, 512]))
                    nc.vector.reciprocal(bcast[:], bcast[:])
                    if j == 0:
                        nc.vector.tensor_mul(
                            yT[pr][0:64, ts], o[0:64, :], bcast[:])
                    else:
                        ybuf = att2.tile([64, 512], BF16, tag="ybuf",
                                         bufs=2, name="ybuf")
                        nc.vector.tensor_mul(
                            ybuf[:], o[0:64, :], bcast[:])
                        nc.gpsimd.dma_start(yT[pr][64:128, ts], ybuf[:])

            def attention(p, proj_feed, pending, extra=None):
                # Software-pipelined: sc[l] is emitted BEFORE attnV[l-1], so
                # the in-order PE stream never sits behind a wait on exp[l]
                # while score work for the next chunk is available. Steady
                # state: ACT runs exp back-to-back; PE interleaves scores,
                # attnV and filler (vn/qkv/proj) between waits that are
                # already satisfied.
                hA, hB = 2 * p, 2 * p + 1
                vaA = v_aug[hA].rearrange("p (n w) -> p n w", w=66)
                vaB = v_aug[hB].rearrange("p (n w) -> p n w", w=66)
                for t in range(nTc):
                    ts = slice(t * 512, (t + 1) * 512)
                    oA = mm2_ps.tile([66, 512], F32, tag="mm2A", name="oA")
                    oB = mm2_ps.tile([66, 512], F32, tag="mm2B", name="oB")
                    feed = proj_feed(t) if proj_feed else []
                    ets = [None, None]
                    pending2 = None
                    for l in range(nL + 1 if pipelined else nL):
                        if l < nL:
                            if extra:
                                extra(t, l)
                            lsl = slice(l * 128, (l + 1) * 128)
                            sc = sc_ps.tile([128, 1024], F32, tag="sc",
                                            name="sc")
                            nc.tensor.matmul(
                                sc[:, 0:512],
                                kcatT[p][0:64, lsl], qnT[p][0:64, ts],
                                start=True, stop=True, tile_position=(0, 0))
                            nc.tensor.matmul(
                                sc[:, 512:1024],
                                kcatT[p][64:128, lsl], qnT[p][64:128, ts],
                                start=True, stop=True, tile_position=(64, 0))
                            et = att2.tile([128, 1024], BF16, tag="exp",
                                           bufs=4, name="et")
                            nc.scalar.activation(et[:], sc[:], AF.Exp,
                                                 scale=float(scale))
                            ets[l % 2] = et
                        if pending and l == 1:
                            pending2 = normalize(*pending)
                            pending = None
                        if pending2 and l == 4:
                            normalize2(*pending2)
                            pending2 = None
                        lm = l - 1 if pipelined else l
                        if 0 <= lm < nL:
                            etm = ets[lm % 2]
                            nc.tensor.matmul(
                                oA[:], vaA[:, lm, :], etm[:, 0:512],
                                start=(lm == 0), stop=(lm == nL - 1))
                            nc.tensor.matmul(
                                oB[:], vaB[:, lm, :], etm[:, 512:1024],
                                start=(lm == 0), stop=(lm == nL - 1))
                            if feed and lm >= 8 and (lm - 8) % 3 == 0:
                                d = (lm - 8) // 3
                                if d < len(feed):
                                    proj_group(*feed[d])
                    pending = (hA, hB, oA, oB, ts)
                return pending

            # ---- phase 1: loads + qkv projection ----
            with tc.tile_pool(name="ph1", bufs=1) as ph1:
                qTr = qT.rearrange("(n p) t -> p n t", p=128)
                wqr = wqkvT.rearrange("(n p) m -> p n m", p=128)
                # DMA order = criticality: W(m=0), q^T, pair-0 k/v loads
                wqm0 = ph1.tile([128, nC, 128], BF16, tag="wqm0")
                nc.sync.dma_start(wqm0[:], wqr[:, :, 0:128])

                qt = ph1.tile([128, nC, T], BF16, tag="qt")
                for k in range(nC):
                    nc.sync.dma_start(qt[:, k:k + 1, :],
                                      qTr[:, k:k + 1, :])

                def load_pair(p, eng=None):
                    # kcat^T kx part (pos pre-added on host) + v_aug XL part
                    eng = eng or nc.sync
                    eng.dma_start(
                        kcatT[p][:, 0:S],
                        kxlT[p * 128:(p + 1) * 128, :])
                    for h in (2 * p, 2 * p + 1):
                        va = v_aug[h].rearrange("p (n w) -> p n w", w=66)
                        eng.dma_start(
                            va[:, :, 64:66],
                            ones_in[:, 0:2 * nL]
                            .rearrange("p (n w) -> p n w", w=2))
                        eng.dma_start(
                            va[:, 0:nS, 0:HD],
                            vxl.rearrange("(n p) d -> p n d", p=128)
                            [:, :, h * HD:(h + 1) * HD])

                load_pair(0)
                wq45 = ph1.tile([128, nC, 2 * 128], BF16, tag="wq45")
                nc.sync.dma_start(wq45[:], wqr[:, :, 512:768])

                def qkv_mt(m, t, pool, wqm):
                    # one (m-tile, t-chunk) piece of the qkv projection
                    p = m % 2
                    ts = slice(t * 512, (t + 1) * 512)
                    ps = pool.tile([128, 512], F32, tag="qkv", name="ps")
                    for k in range(nC):
                        nc.tensor.matmul(
                            ps[:], wqm[:, k, :], qt[:, k, ts],
                            start=(k == 0), stop=(k == nC - 1))
                    dst = (qnT[p][:, ts] if m < 2 else
                           kcatT[p][:, S + t * 512:S + (t + 1) * 512])
                    nc.vector.tensor_copy(dst, ps[:])

                def qkv_m(m, pool, wqm=None):
                    # one 128-row m-tile of the qkv projection
                    if wqm is None:
                        wqm = ph1.tile([128, nC, 128], BF16, tag="wqm",
                                       bufs=1, name="wqm")
                        nc.sync.dma_start(
                            wqm[:], wqr[:, :, m * 128:(m + 1) * 128])
                    for t in range(nTc):
                        qkv_mt(m, t, pool, wqm)

                if stage != "dma":
                    with tc.tile_pool(name="qkv1", bufs=3,
                                      space="PSUM") as qkv1:
                        qkv_m(0, qkv1, wqm=wqm0)   # qn^T pair 0
                        qkv_m(2, qkv1)   # kn^T pair 0

                # attention psum pools (outlive ph1; LIFO within PSUM is
                # independent of the SBUF pool stack)
                sc_ps = ctx.enter_context(
                    tc.tile_pool(name="sc_ps", bufs=2, space="PSUM"))
                mm2_ps = ctx.enter_context(
                    tc.tile_pool(name="mm2_ps", bufs=1, space="PSUM"))

                # pair-1 qkv m-tiles drip-fed into pair-0's ACT-bound
                # loops (full stage): schedule (t, l) -> (m-tile, t-chunk)
                drip_sched = {(0, 21): (1, 0), (0, 27): (3, 0),
                              (1, 5): (1, 1), (1, 15): (3, 1),
                              (1, 25): (1, 2), (2, 5): (3, 2),
                              (2, 15): (1, 3), (2, 25): (3, 3)}
                if stage == "full":
                    wqm13 = {}
                    for m in (1, 3):
                        w = ph1.tile([128, nC, 128], BF16, tag=f"wqm{m}",
                                     name=f"wqm{m}")
                        nc.sync.dma_start(
                            w[:], wqr[:, :, m * 128:(m + 1) * 128])
                        wqm13[m] = w
                    load_pair(1, eng=nc.gpsimd)
                    qkvd = ctx.enter_context(
                        tc.tile_pool(name="qkvd", bufs=1, space="PSUM"))

                    def qkv_piece(m, tq):
                        tsq = slice(tq * 512, (tq + 1) * 512)
                        ps = qkvd.tile([128, 512], F32, tag="qkvd",
                                       name="psd")
                        for k in range(nC):
                            nc.tensor.matmul(
                                ps[:], wqm13[m][:, k, :], qt[:, k, tsq],
                                start=(k == 0), stop=(k == nC - 1))
                        dst = (qnT[1][:, tsq] if m < 2 else
                               kcatT[1][:, S + tq * 512:S + (tq + 1) * 512])
                        nc.vector.tensor_copy(dst, ps[:])

                # vn for all 4 heads, directly in [t, dim] layout, computed
                # chunk-by-chunk inside pair-0 tchunk-0's l-loop: group j is
                # written at iteration j and first read at iteration 16+j
                with tc.tile_pool(name="vnp", bufs=1, space="PSUM") as vnp:
                    def vn_extra(t, l):
                        if stage == "full" and (t, l) in drip_sched:
                            qkv_piece(*drip_sched[(t, l)])
                        # half-rate spread: vn[i] at l=2i keeps per-iter PE
                        # load under the ACT exp rate; vn[i] is first read by
                        # attnV at iteration nS+i+1 > 2i for all i < nT
                        if t != 0 or l % 2 != 0 or l // 2 >= nT:
                            return
                        i = l // 2
                        vn = vnp.tile([128, 256], F32, tag="vn", name="vn")
                        for k in range(nC):
                            nc.tensor.matmul(
                                vn[:],
                                qt[:, k, i * 128:(i + 1) * 128],
                                wq45[:, k, :],
                                start=(k == 0), stop=(k == nC - 1))
                        for h in range(HPC):
                            va = v_aug[h].rearrange("p (n w) -> p n w", w=66)
                            nc.vector.tensor_copy(
                                va[:, nS + i, 0:HD],
                                vn[:, h * HD:(h + 1) * HD])

                    # pair-0 attention; remaining qkv m-tiles emitted after
                    # it fill PE slack under the ACT-bound loop
                    if stage in ("attn1", "attn2", "attn2b", "full"):
                        pending = attention(0, None, None, extra=vn_extra)
                    elif stage == "qkv":
                        for i in range(nT):
                            vn_extra(0, 2 * i)

                if stage != "full":
                    load_pair(1, eng=nc.gpsimd)
                if stage not in ("dma", "full"):
                    with tc.tile_pool(name="qkv2", bufs=1,
                                      space="PSUM") as qkv2:
                        qkv_m(1, qkv2)   # qn^T pair 1
                        qkv_m(3, qkv2)   # kn^T pair 1
                if stage == "attn2b":
                    # pair-1 attention emitted inside ph1 (same context as
                    # pair-0), no drip
                    pending = attention(1, None, pending)
                if stage in ("dma", "qkv"):
                    # emulate output traffic from qt (timing only)
                    for d in range(nC):
                        nc.sync.dma_start(outT[d * 128:(d + 1) * 128, :],
                                          qt[:, d, :])

            # ---- pair-1 attention with drip-fed output projection ----
            with tc.tile_pool(name="tail_sb", bufs=1) as tail, \
                 tc.tile_pool(name="pj_ps", bufs=1, space="PSUM") as pj_ps:
                wp = tail.tile([128, 2 * C], BF16, tag="wp")
                nc.sync.dma_start(wp[:], wp4[:])

                def proj_group(t, d, heads=tuple(range(HPC))):
                    # out^T[d-chunk, tchunk t]: one K=128 matmul per head
                    # pair (paired y layout)
                    ts = slice(t * 512, (t + 1) * 512)
                    pairs = sorted({h // 2 for h in heads})
                    ps = pj_ps.tile([128, 512], F32, tag="proj", name="pj")
                    for pr in pairs:
                        nc.tensor.matmul(
                            ps[:],
                            wp[:, pr * C + d * 128:pr * C + (d + 1) * 128],
                            yT[pr][:, ts],
                            start=(pr == pairs[0]), stop=(pr == pairs[-1]),
                            tile_position=(0, 0))
                    ob = tail.tile([128, 512], BF16, tag="ob", bufs=4,
                                   name="ob")
                    nc.vector.tensor_copy(ob[:], ps[:])
                    # SP is idle once the prologue streams finish; alternate
                    # output-drip DMAs between the two free queues
                    eng = nc.gpsimd if d % 2 == 0 else nc.sync
                    eng.dma_start(outT[d * 128:(d + 1) * 128, ts], ob[:])

                def feed(t):
                    # during pair-1 tchunk t, project tchunk t-1
                    if t == 0:
                        return []
                    return [(t - 1, d) for d in range(nC)]

                if stage == "full":
                    pending = attention(1, feed, pending)
                    normalize2(*normalize(*pending))
                    for d in range(nC):
                        proj_group(nTc - 1, d)
                elif stage == "attn1":
                    normalize2(*normalize(*pending))
                    for t in range(nTc):
                        for d in range(nC):
                            proj_group(t, d, heads=(0, 1))
                elif stage == "attn2":
                    # pair-1 attention without the proj drip; proj at end
                    pending = attention(1, None, pending)
                    normalize2(*normalize(*pending))
                    for t in range(nTc):
                        for d in range(nC):
                            proj_group(t, d)
                elif stage == "attn2b":
                    # pair-1 ran inside ph1; only normalize+proj here
                    normalize2(*normalize(*pending))
                    for t in range(nTc):
                        for d in range(nC):
                            proj_group(t, d)
        if timing_mode:
            with tc.tile_pool(name="tm_sb", bufs=1) as tmp:
                tt = tmp.tile([128, 128], F32, tag="tiny")
                nc.sync.dma_start(tt[:], dummy[:])
                nc.sync.dma_start(tiny[:], tt[:])

    nc.compile()
    return nc


_cache = {}


def _program(T, S, C):
    key = (T, S, C)
    if key not in _cache:
        _cache[key] = build_program(T, S, C)
    return _cache[key]


def core_inputs(q, k_xl, v_xl, W_qkv, W_proj, pos_emb, core):
    """Host-side shard prep for one core (slicing + layout transposes +
    bf16 conversion; pos_emb is pre-added to the k_xl cache here)."""
    import ml_dtypes
    bf = ml_dtypes.bfloat16
    C = q.shape[2]
    b = core // 4
    h0 = (core % 4) * HPC
    cols = slice(h0 * HD, (h0 + HPC) * HD)
    rows = np.r_[h0 * HD:(h0 + HPC) * HD]
    wrows = np.concatenate([rows, C + rows, 2 * C + rows])
    Wt = W_proj[:, cols].T.reshape(HPC, HD, C)
    wp4 = np.concatenate(
        [np.concatenate([Wt[2 * p], Wt[2 * p + 1]], axis=0)
         for p in range(2)], axis=1)
    return {
        "qT": np.ascontiguousarray(q[b].T).astype(bf),
        "wqkvT": np.ascontiguousarray(W_qkv[wrows].T).astype(bf),
        "wp4": np.ascontiguousarray(wp4).astype(bf),
        "kxlT": np.ascontiguousarray(k_xl[b].T[cols]
                                     + pos_emb.T[cols]).astype(bf),
        "vxl": np.ascontiguousarray(v_xl[b][:, cols]).astype(bf),
        "ones": np.ones(
            (128, max(64, 2 * (q.shape[1] + k_xl.shape[1]) // 128)), bf),
    }


def kernel(q, k_xl, v_xl, W_qkv, W_proj, pos_emb, is_causal):
    q = np.asarray(q, np.float32)
    k_xl = np.asarray(k_xl, np.float32)
    v_xl = np.asarray(v_xl, np.float32)
    W_qkv = np.asarray(W_qkv, np.float32)
    W_proj = np.asarray(W_proj, np.float32)
    pos_emb = np.asarray(pos_emb, np.float32)
    B, T, C = q.shape
    S = k_xl.shape[1]

    nc = _program(T, S, C)
    in_maps = [core_inputs(q, k_xl, v_xl, W_qkv, W_proj, pos_emb, c)
               for c in range(N_CORES)]
    res = run_bass_kernel_spmd(nc, in_maps, list(range(N_CORES)))

    out = np.zeros((B, T, C), np.float32)
    for c in range(N_CORES):
        out[c // 4] += res.results[c]["outT"].T.astype(np.float32)
    return out



# revision 43
# speedup vs baseline: 1.1170x; 1.1170x over previous
"""Trainium2 Bass kernel for nn_Attention_XL (B=2,T=2048,C=1024,S=2048,H=16).

Sharding: (batch, head) pairs across 8 cores — core c handles batch c//4 and
heads [(c%4)*4, (c%4)*4+4). QKV projection is column-sharded by head (no
redundant FLOPs); W_proj is column-sharded; per-core partial outputs are
summed on the host (the tensor-parallel unshard step).

All DMA-facing tensors are bf16 (host converts; pos_emb is pre-added into
the k_xl cache on the host): measured DMA bandwidth into SBUF collapses
~7x while the engines are busy, so halving bytes moved is worth far more
than it costs in precision (rel err 6e-3 vs the 2e-2 gate).

Per-core device program (feature-on-partition, i.e. transposed; matmul
operands bf16, PSUM fp32):
  kcat^T = [kx^T (pos pre-added), kn^T]; vn computed directly in [t, dim]
  layout into v_aug, whose ones columns later yield the softmax
  denominator rows.
  Attention per head pair, flash-style over key chunks, software
  pipelined: sc[l] is emitted before attnV[l-1] so the in-order PE stream
  never waits on exp[l] with score work available; ACT (exp, [128,1024]
  per chunk, scale folded in) is the pace-setting engine. et ring bufs=4
  lets ACT run ahead of attnV.
  Chunk-boundary normalization: the PSUM accumulators are copied to SBUF
  immediately (a K=1 broadcast matmul here stalls the PE ~5us, and gpsimd
  partition_broadcast returns zeros on this toolchain, so the sumexp row
  is partition-broadcast via a DRAM-roundtrip DMA on the gpsimd queue);
  reciprocal+scale then run off the critical path.
  Heads are paired into [128,T] y tiles (head B hops partitions via a
  small SBUF->SBUF DMA) so the output projection contracts K=128 in two
  matmuls per tile instead of four K=64 ones.
  Pair-1's QKV m-tiles and the output projection are drip-fed into the
  ACT-bound l-loops (schedule in drip_sched / proj feed); pair-1's XL
  loads and all mid-loop DMAs ride the gpsimd SWDGE queue to stay off the
  SP queue that serves the prologue streams.

  Measurement notes (axon-tunneled cores): per-call pipelined slopes are
  dominated by ~0.5-0.8ms of submission overhead; true device time comes
  from the marginal cost between repeat=1 and repeat=5 bodies
  (bench_repeat.py). The shared host's speed drifts between runs - only
  back-to-back A/B comparisons are trustworthy.
"""
import sys

sys.path.insert(0, "/opt/trn_rl_repo")

import numpy as np
import concourse.bass as bass
import concourse.bacc as bacc
import concourse.mybir as mybir
import concourse.tile as tile
from concourse.bass_utils import run_bass_kernel_spmd

F32 = mybir.dt.float32
F32R = mybir.dt.float32r
BF16 = mybir.dt.bfloat16
AF = mybir.ActivationFunctionType

HD = 64          # head dim
HPC = 4          # heads per core
N_CORES = 8


def r(ap):
    return ap.bitcast(F32R)


def build_program(T, S, C, timing_mode=False, stage="full", repeat=1,
                  pipelined=True):
    """Build + compile the per-core Bass program. Same program on all cores.

    timing_mode: big I/O tensors become Internal DRAM (no host transfer) so
    pipelined wall-clock isolates device exec; compute is unchanged.
    stage: 'full' | 'attn1' (pair-0 attention only) | 'qkv' (no attention) |
    'dma' (loads only) — timing diagnostics, values meaningless off 'full'.
    repeat: emit the whole body k times (timing: marginal cost per repeat
    isolates device exec from per-call submission overhead).
    pipelined: emit attnV[l-1] after sc[l] (software pipelining) vs legacy
    same-iteration order."""
    L = S + T
    nL = L // 128           # key chunks
    nS = S // 128
    nT = T // 128
    nC = C // 128           # contraction chunks for qkv proj
    nTc = T // 512          # 512-wide t chunks
    R = 3 * HPC * HD        # rows of W' (768)
    scale = 1.0 / np.sqrt(HD)

    nc = bacc.Bacc("TRN2", target_bir_lowering=False, debug=False)

    ik = "Internal" if timing_mode else "ExternalInput"
    ok = "Internal" if timing_mode else "ExternalOutput"
    qT = nc.dram_tensor("qT", [C, T], BF16, kind=ik).ap()
    wqkvT = nc.dram_tensor("wqkvT", [C, R], BF16, kind=ik).ap()
    wp4 = nc.dram_tensor("wp4", [128, 2 * C], BF16, kind=ik).ap()
    kxlT = nc.dram_tensor("kxlT", [HPC * HD, S], BF16, kind=ik).ap()
    vxl = nc.dram_tensor("vxl", [S, HPC * HD], BF16, kind=ik).ap()
    n_ones = max(64, 2 * (S + T) // 128)
    ones_in = nc.dram_tensor("ones", [128, n_ones], BF16, kind=ik).ap()
    outT = nc.dram_tensor("outT", [C, T], BF16, kind=ok).ap()
    nscratch = nc.dram_tensor("nscratch", [2, 512], F32, kind="Internal").ap()
    if timing_mode:
        dummy = nc.dram_tensor("tm_in", [128, 128], F32,
                               kind="ExternalInput").ap()
        tiny = nc.dram_tensor("tm_out", [128, 128], F32,
                              kind="ExternalOutput").ap()

    with tile.TileContext(nc) as tc:
        import contextlib
        for _rep in range(repeat):
          with contextlib.ExitStack() as ctx:
            persist = ctx.enter_context(tc.tile_pool(name="persist", bufs=1))
            vaugp = ctx.enter_context(tc.tile_pool(name="vaugp", bufs=1))
            att2 = ctx.enter_context(tc.tile_pool(name="att_sb", bufs=1))

            qnT = [persist.tile([128, T], BF16, tag=f"qnT{p}", name=f"qnT{p}")
                   for p in range(2)]
            kcatT = [persist.tile([128, L], BF16, tag=f"kcatT{p}",
                                  name=f"kcatT{p}") for p in range(2)]
            yT = [persist.tile([128, T], BF16, tag=f"yTp{p}",
                  name=f"yTp{p}") for p in range(2)]
            v_aug = [vaugp.tile([128, nL * 66], BF16, tag=f"vaug{h}",
                                name=f"vaug{h}") for h in range(HPC)]

            # psum pools for the attention loops are entered mid-build
            # (after the big qkv pool closes); declared here for closures
            sc_ps = mm2_ps = pj_ps = None
            proj_group = None

            def normalize(hA, hB, oA, oB, ts):
                # phase 1 (chunk boundary): copy the PSUM accumulators to
                # SBUF so the next chunk's attnV start=True isn't gated on
                # the whole normalization chain. A K=1 broadcast matmul here
                # measurably stalls the PE (~5us each) - avoided entirely.
                ocpA = att2.tile([66, 512], F32, tag="ocp", bufs=4,
                                 name="ocpA")
                nc.vector.tensor_copy(r(ocpA[:]), oA[:])
                ocpB = att2.tile([66, 512], F32, tag="ocp", bufs=4,
                                 name="ocpB")
                nc.vector.tensor_copy(r(ocpB[:]), oB[:])
                return (hA, hB, ocpA, ocpB, ts)

            def normalize2(hA, hB, oA, oB, ts):
                # phase 2 (deferred): y^T = out^T * (1/sumexp). The sumexp
                # row is partition-broadcast via a DRAM-roundtrip DMA (the
                # K=1 matmul alternative stalls the PE ~5us; gpsimd
                # partition_broadcast returns zeros on this toolchain).
                # Heads are paired into one [128,T] y tile so the output
                # projection contracts K=128: head A lands in rows 0:64
                # directly; head B goes through a temp and a repartitioning
                # SBUF->SBUF DMA into rows 64:128 (DVE can't cross
                # partitions).
                pr = hA // 2
                for j, (h, o) in enumerate(((hA, oA), (hB, oB))):
                    nc.gpsimd.dma_start(nscratch[j:j + 1, :], o[64:65, :])
                    bcast = att2.tile([64, 512], F32, tag="bcast", bufs=2,
                                      name="bcast")
                    nc.gpsimd.dma_start(
                        bcast[:], nscratch[j:j + 1, :].broadcast_to([64, 512]))
                    nc.vector.reciprocal(bcast[:], bcast[:])
                    if j == 0:
                        nc.vector.tensor_mul(
                            yT[pr][0:64, ts], o[0:64, :], bcast[:])
                    else:
                        ybuf = att2.tile([64, 512], BF16, tag="ybuf",
                                         bufs=2, name="ybuf")
                        nc.vector.tensor_mul(
                            ybuf[:], o[0:64, :], bcast[:])
                        nc.gpsimd.dma_start(yT[pr][64:128, ts], ybuf[:])

            def attention(p, proj_feed, pending, extra=None):
                # Software-pipelined: sc[l] is emitted BEFORE attnV[l-1], so
                # the in-order PE stream never sits behind a wait on exp[l]
                # while score work for the next chunk is available. Steady
                # state: ACT runs exp back-to-back; PE interleaves scores,
                # attnV and filler (vn/qkv/proj) between waits that are
                # already satisfied.
                hA, hB = 2 * p, 2 * p + 1
                vaA = v_aug[hA].rearrange("p (n w) -> p n w", w=66)
                vaB = v_aug[hB].rearrange("p (n w) -> p n w", w=66)
                for t in range(nTc):
                    ts = slice(t * 512, (t + 1) * 512)
                    oA = mm2_ps.tile([66, 512], F32, tag="mm2A", name="oA")
                    oB = mm2_ps.tile([66, 512], F32, tag="mm2B", name="oB")
                    feed = proj_feed(t) if proj_feed else []
                    ets = [None, None]
                    pending2 = None
                    for l in range(nL + 1 if pipelined else nL):
                        if l < nL:
                            if extra:
                                extra(t, l)
                            lsl = slice(l * 128, (l + 1) * 128)
                            sc = sc_ps.tile([128, 1024], F32, tag="sc",
                                            name="sc")
                            nc.tensor.matmul(
                                sc[:, 0:512],
                                kcatT[p][0:64, lsl], qnT[p][0:64, ts],
                                start=True, stop=True, tile_position=(0, 0))
                            nc.tensor.matmul(
                                sc[:, 512:1024],
                                kcatT[p][64:128, lsl], qnT[p][64:128, ts],
                                start=True, stop=True, tile_position=(64, 0))
                            et = att2.tile([128, 1024], BF16, tag="exp",
                                           bufs=4, name="et")
                            nc.scalar.activation(et[:], sc[:], AF.Exp,
                                                 scale=float(scale))
                            ets[l % 2] = et
                        if pending and l == 1:
                            pending2 = normalize(*pending)
                            pending = None
                        if pending2 and l == 4:
                            normalize2(*pending2)
                            pending2 = None
                        lm = l - 1 if pipelined else l
                        if 0 <= lm < nL:
                            etm = ets[lm % 2]
                            nc.tensor.matmul(
                                oA[:], vaA[:, lm, :], etm[:, 0:512],
                                start=(lm == 0), stop=(lm == nL - 1))
                            nc.tensor.matmul(
                                oB[:], vaB[:, lm, :], etm[:, 512:1024],
                                start=(lm == 0), stop=(lm == nL - 1))
                            if feed and lm >= 8 and (lm - 8) % 3 == 0:
                                d = (lm - 8) // 3
                                if d < len(feed):
                                    proj_group(*feed[d])
                    pending = (hA, hB, oA, oB, ts)
                return pending

            # ---- phase 1: loads + qkv projection ----
            with tc.tile_pool(name="ph1", bufs=1) as ph1:
                qTr = qT.rearrange("(n p) t -> p n t", p=128)
                wqr = wqkvT.rearrange("(n p) m -> p n m", p=128)
                # DMA order = criticality: W(m=0), q^T, pair-0 k/v loads
                wqm0 = ph1.tile([128, nC, 128], BF16, tag="wqm0")
                nc.sync.dma_start(wqm0[:], wqr[:, :, 0:128])

                qt = ph1.tile([128, nC, T], BF16, tag="qt")
                for k in range(nC):
                    nc.sync.dma_start(qt[:, k:k + 1, :],
                                      qTr[:, k:k + 1, :])

                def load_pair(p, eng=None):
                    # kcat^T kx part (pos pre-added on host) + v_aug XL part
                    eng = eng or nc.sync
                    eng.dma_start(
                        kcatT[p][:, 0:S],
                        kxlT[p * 128:(p + 1) * 128, :])
                    for h in (2 * p, 2 * p + 1):
                        va = v_aug[h].rearrange("p (n w) -> p n w", w=66)
                        eng.dma_start(
                            va[:, :, 64:66],
                            ones_in[:, 0:2 * nL]
                            .rearrange("p (n w) -> p n w", w=2))
                        eng.dma_start(
                            va[:, 0:nS, 0:HD],
                            vxl.rearrange("(n p) d -> p n d", p=128)
                            [:, :, h * HD:(h + 1) * HD])

                load_pair(0)
                wq45 = ph1.tile([128, nC, 2 * 128], BF16, tag="wq45")
                nc.sync.dma_start(wq45[:], wqr[:, :, 512:768])

                def qkv_mt(m, t, pool, wqm):
                    # one (m-tile, t-chunk) piece of the qkv projection
                    p = m % 2
                    ts = slice(t * 512, (t + 1) * 512)
                    ps = pool.tile([128, 512], F32, tag="qkv", name="ps")
                    for k in range(nC):
                        nc.tensor.matmul(
                            ps[:], wqm[:, k, :], qt[:, k, ts],
                            start=(k == 0), stop=(k == nC - 1))
                    dst = (qnT[p][:, ts] if m < 2 else
                           kcatT[p][:, S + t * 512:S + (t + 1) * 512])
                    nc.vector.tensor_copy(dst, ps[:])

                def qkv_m(m, pool, wqm=None):
                    # one 128-row m-tile of the qkv projection
                    if wqm is None:
                        wqm = ph1.tile([128, nC, 128], BF16, tag="wqm",
                                       bufs=1, name="wqm")
                        nc.sync.dma_start(
                            wqm[:], wqr[:, :, m * 128:(m + 1) * 128])
                    for t in range(nTc):
                        qkv_mt(m, t, pool, wqm)

                if stage != "dma":
                    with tc.tile_pool(name="qkv1", bufs=3,
                                      space="PSUM") as qkv1:
                        qkv_m(0, qkv1, wqm=wqm0)   # qn^T pair 0
                        qkv_m(2, qkv1)   # kn^T pair 0

                # attention psum pools (outlive ph1; LIFO within PSUM is
                # independent of the SBUF pool stack)
                sc_ps = ctx.enter_context(
                    tc.tile_pool(name="sc_ps", bufs=2, space="PSUM"))
                mm2_ps = ctx.enter_context(
                    tc.tile_pool(name="mm2_ps", bufs=1, space="PSUM"))

                # pair-1 qkv m-tiles drip-fed into pair-0's ACT-bound
                # loops (full stage): schedule (t, l) -> (m-tile, t-chunk)
                drip_sched = {(0, 21): (1, 0), (0, 27): (3, 0),
                              (1, 5): (1, 1), (1, 15): (3, 1),
                              (1, 25): (1, 2), (2, 5): (3, 2),
                              (2, 15): (1, 3), (2, 25): (3, 3)}
                if stage == "full":
                    wqm13 = {}
                    for m in (1, 3):
                        w = ph1.tile([128, nC, 128], BF16, tag=f"wqm{m}",
                                     name=f"wqm{m}")
                        nc.sync.dma_start(
                            w[:], wqr[:, :, m * 128:(m + 1) * 128])
                        wqm13[m] = w
                    load_pair(1, eng=nc.gpsimd)
                    qkvd = ctx.enter_context(
                        tc.tile_pool(name="qkvd", bufs=1, space="PSUM"))

                    def qkv_piece(m, tq):
                        tsq = slice(tq * 512, (tq + 1) * 512)
                        ps = qkvd.tile([128, 512], F32, tag="qkvd",
                                       name="psd")
                        for k in range(nC):
                            nc.tensor.matmul(
                                ps[:], wqm13[m][:, k, :], qt[:, k, tsq],
                                start=(k == 0), stop=(k == nC - 1))
                        dst = (qnT[1][:, tsq] if m < 2 else
                               kcatT[1][:, S + tq * 512:S + (tq + 1) * 512])
                        nc.vector.tensor_copy(dst, ps[:])

                # vn for all 4 heads, directly in [t, dim] layout, computed
                # chunk-by-chunk inside pair-0 tchunk-0's l-loop: group j is
                # written at iteration j and first read at iteration 16+j
                with tc.tile_pool(name="vnp", bufs=1, space="PSUM") as vnp:
                    def vn_extra(t, l):
                        if stage == "full" and (t, l) in drip_sched:
                            qkv_piece(*drip_sched[(t, l)])
                        # half-rate spread: vn[i] at l=2i keeps per-iter PE
                        # load under the ACT exp rate; vn[i] is first read by
                        # attnV at iteration nS+i+1 > 2i for all i < nT
                        if t != 0 or l % 2 != 0 or l // 2 >= nT:
                            return
                        i = l // 2
                        vn = vnp.tile([128, 256], F32, tag="vn", name="vn")
                        for k in range(nC):
                            nc.tensor.matmul(
                                vn[:],
                                qt[:, k, i * 128:(i + 1) * 128],
                                wq45[:, k, :],
                                start=(k == 0), stop=(k == nC - 1))
                        for h in range(HPC):
                            va = v_aug[h].rearrange("p (n w) -> p n w", w=66)
                            nc.vector.tensor_copy(
                                va[:, nS + i, 0:HD],
                                vn[:, h * HD:(h + 1) * HD])

                    # pair-0 attention; remaining qkv m-tiles emitted after
                    # it fill PE slack under the ACT-bound loop
                    if stage in ("attn1", "attn2", "attn2b", "full"):
                        pending = attention(0, None, None, extra=vn_extra)
                    elif stage == "qkv":
                        for i in range(nT):
                            vn_extra(0, 2 * i)

                if stage != "full":
                    load_pair(1, eng=nc.gpsimd)
                if stage not in ("dma", "full"):
                    with tc.tile_pool(name="qkv2", bufs=1,
                                      space="PSUM") as qkv2:
                        qkv_m(1, qkv2)   # qn^T pair 1
                        qkv_m(3, qkv2)   # kn^T pair 1
                if stage == "attn2b":
                    # pair-1 attention emitted inside ph1 (same context as
                    # pair-0), no drip
                    pending = attention(1, None, pending)
                if stage in ("dma", "qkv"):
                    # emulate output traffic from qt (timing only)
                    for d in range(nC):
                        nc.sync.dma_start(outT[d * 128:(d + 1) * 128, :],
                                          qt[:, d, :])

            # ---- pair-1 attention with drip-fed output projection ----
            with tc.tile_pool(name="tail_sb", bufs=1) as tail, \
                 tc.tile_pool(name="pj_ps", bufs=1, space="PSUM") as pj_ps:
                wp = tail.tile([128, 2 * C], BF16, tag="wp")
                nc.sync.dma_start(wp[:], wp4[:])

                def proj_group(t, d, heads=tuple(range(HPC))):
                    # out^T[d-chunk, tchunk t]: one K=128 matmul per head
                    # pair (paired y layout)
                    ts = slice(t * 512, (t + 1) * 512)
                    pairs = sorted({h // 2 for h in heads})
                    ps = pj_ps.tile([128, 512], F32, tag="proj", name="pj")
                    for pr in pairs:
                        nc.tensor.matmul(
                            ps[:],
                            wp[:, pr * C + d * 128:pr * C + (d + 1) * 128],
                            yT[pr][:, ts],
                            start=(pr == pairs[0]), stop=(pr == pairs[-1]),
                            tile_position=(0, 0))
                    ob = tail.tile([128, 512], BF16, tag="ob", bufs=4,
                                   name="ob")
                    nc.vector.tensor_copy(ob[:], ps[:])
                    # SP is idle once the prologue streams finish; alternate
                    # output-drip DMAs between the two free queues
                    eng = nc.gpsimd if d % 2 == 0 else nc.sync
                    eng.dma_start(outT[d * 128:(d + 1) * 128, ts], ob[:])

                def feed(t):
                    # during pair-1 tchunk t, project tchunk t-1
                    if t == 0:
                        return []
                    return [(t - 1, d) for d in range(nC)]

                if stage == "full":
                    pending = attention(1, feed, pending)
                    normalize2(*normalize(*pending))
                    for d in range(nC):
                        proj_group(nTc - 1, d)
                elif stage == "attn1":
                    normalize2(*normalize(*pending))
                    for t in range(nTc):
                        for d in range(nC):
                            proj_group(t, d, heads=(0, 1))
                elif stage == "attn2":
                    # pair-1 attention without the proj drip; proj at end
                    pending = attention(1, None, pending)
                    normalize2(*normalize(*pending))
                    for t in range(nTc):
                        for d in range(nC):
                            proj_group(t, d)
                elif stage == "attn2b":
                    # pair-1 ran inside ph1; only normalize+proj here
                    normalize2(*normalize(*pending))
                    for t in range(nTc):
                        for d in range(nC):
                            proj_group(t, d)
        if timing_mode:
            with tc.tile_pool(name="tm_sb", bufs=1) as tmp:
                tt = tmp.tile([128, 128], F32, tag="tiny")
                nc.sync.dma_start(tt[:], dummy[:])
                nc.sync.dma_start(tiny[:], tt[:])

    nc.compile()
    return nc


_cache = {}


def _program(T, S, C):
    key = (T, S, C)
    if key not in _cache:
        _cache[key] = build_program(T, S, C)
    return _cache[key]


def core_inputs(q, k_xl, v_xl, W_qkv, W_proj, pos_emb, core):
    """Host-side shard prep for one core (slicing + layout transposes +
    bf16 conversion; pos_emb is pre-added to the k_xl cache here)."""
    import ml_dtypes
    bf = ml_dtypes.bfloat16
    C = q.shape[2]
    b = core // 4
    h0 = (core % 4) * HPC
    cols = slice(h0 * HD, (h0 + HPC) * HD)
    rows = np.r_[h0 * HD:(h0 + HPC) * HD]
    wrows = np.concatenate([rows, C + rows, 2 * C + rows])
    Wt = W_proj[:, cols].T.reshape(HPC, HD, C)
    wp4 = np.concatenate(
        [np.concatenate([Wt[2 * p], Wt[2 * p + 1]], axis=0)
         for p in range(2)], axis=1)
    return {
        "qT": np.ascontiguousarray(q[b].T).astype(bf),
        "wqkvT": np.ascontiguousarray(W_qkv[wrows].T).astype(bf),
        "wp4": np.ascontiguousarray(wp4).astype(bf),
        "kxlT": np.ascontiguousarray(k_xl[b].T[cols]
                                     + pos_emb.T[cols]).astype(bf),
        "vxl": np.ascontiguousarray(v_xl[b][:, cols]).astype(bf),
        "ones": np.ones(
            (128, max(64, 2 * (q.shape[1] + k_xl.shape[1]) // 128)), bf),
    }


def kernel(q, k_xl, v_xl, W_qkv, W_proj, pos_emb, is_causal):
    q = np.asarray(q, np.float32)
    k_xl = np.asarray(k_xl, np.float32)
    v_xl = np.asarray(v_xl, np.float32)
    W_qkv = np.asarray(W_qkv, np.float32)
    W_proj = np.asarray(W_proj, np.float32)
    pos_emb = np.asarray(pos_emb, np.float32)
    B, T, C = q.shape
    S = k_xl.shape[1]

    nc = _program(T, S, C)
    in_maps = [core_inputs(q, k_xl, v_xl, W_qkv, W_proj, pos_emb, c)
               for c in range(N_CORES)]
    res = run_bass_kernel_spmd(nc, in_maps, list(range(N_CORES)))

    out = np.zeros((B, T, C), np.float32)
    for c in range(N_CORES):
        out[c // 4] += res.results[c]["outT"].T.astype(np.float32)
    return out

